# revision 59
# baseline (speedup 1.0000x reference)
"""Causal self-attention (GQA + RoPE) Trainium2 Bass kernel, 8-way sharded.

Sharding: core c -> batch b = c // 2, head-half hh = c % 2.
Each core computes the qkv projection, attention and output projection for
its batch and its 16 query heads / 4 kv heads (kv groups kept whole); the
output projection is a row-shard of Wproj, so the two cores of a batch
produce partial sums that the host adds.

Device-side layout tricks (host prepares):
  - all matmul operands are bf16 (halves HBM traffic; PE rate is identical
    to f32r for >=256-wide moving data, and strictly better below that).
  - x is fed pre-transposed (xT [C, T]) so the qkv matmul needs no on-device
    transpose of x.
  - per-core qkv columns are reordered to [k, v, q0..15] (k/v first so the
    attention inputs finalize early), with Wq/Wk columns de-interleaved per
    head (even rope pairs then odd) so RoPE is the rotate-half form with
    free-dim slices only.
  - scores are computed transposed (scoresT = k_tile^T-matmul) so the
    attention-weights matmul needs no transposes; the softmax denominator
    comes from a 128-wide all-ones stationary matmul (every output partition
    gets the partition-sum), so no extra broadcast pass is needed.
"""

import os

os.environ.setdefault("JAX_PLATFORMS", "axon")

import numpy as np
import ml_dtypes

BF16 = ml_dtypes.bfloat16

B, T, C = 4, 1024, 4096
H, KV, HD = 32, 8, 128
REP = H // KV  # 4

NQ = 16      # q heads per core
NKV = 4      # kv heads per core
COLS = (NQ + 2 * NKV) * HD   # 3072 local qkv cols: k0..3 v0..3 q0..15
NTT = T // 128               # 8 token tiles
SCALE = float(1.0 / np.sqrt(np.float32(HD)).astype(np.float32))

# 512-col slot -> first qkT head index (slot 1 is v, spilled untransposed)
SLOT_HEAD = {0: 16, 2: 0, 3: 4, 4: 8, 5: 12}

_CACHE: dict = {}


def _build_nc():
    import concourse.mybir as mybir
    import concourse.tile as tile
    from concourse import bacc
    from concourse.bass import ts
    from concourse.masks import make_identity

    f32 = mybir.dt.float32
    bf16 = mybir.dt.bfloat16
    Exp = mybir.ActivationFunctionType.Exp

    nc = bacc.Bacc(None, target_bir_lowering=False, debug=False)

    xT_d = nc.dram_tensor("xT", [C, T], bf16, kind="ExternalInput")
    # [colhalf, chalf, j(512-col chunk), cc(128-row chunk), 128, 512]
    wqkv_d = nc.dram_tensor("wqkv", [2, 2, 3, 16, 128, 512], bf16, kind="ExternalInput")
    # [ccol(512-col chunk), ycc(128-row chunk), 128, 512]
    wproj_d = nc.dram_tensor("wproj", [8, 16, 128, 512], bf16, kind="ExternalInput")
    cos_d = nc.dram_tensor("cosn", [T, 64], f32, kind="ExternalInput")
    sin_d = nc.dram_tensor("sinn", [T, 64], f32, kind="ExternalInput")
    # compact diagonal-pair mask: [p, 0:256] = (p <= t), [p, 256:384] =
    # (p <= t-128) for the half-width second diagonal block
    masks_d = nc.dram_tensor("masks", [128, 384], bf16, kind="ExternalInput")
    out_d = nc.dram_tensor("out", [T, C], f32, kind="ExternalOutput")
    # scratch: q/k transposed [head, hd=128, T] (0..15 q, 16..19 k); v natural
    qkT_d = nc.dram_tensor("qkT_scratch", [NQ + NKV, 128, T], bf16)
    v_d = nc.dram_tensor("v_scratch", [T, NKV * HD], bf16)

    with (
        tile.TileContext(nc) as tc,
        tc.tile_pool(name="const", bufs=1) as const_p,
        tc.tile_pool(name="vsb", bufs=1) as vsb_p,
        tc.tile_pool(name="msk", bufs=1) as msk_p,
        tc.tile_pool(name="qt", bufs=4) as qt_p,
        tc.tile_pool(name="kt", bufs=2) as kt_p,
        tc.tile_pool(name="rope", bufs=3) as rope_p,
        tc.tile_pool(name="tstage", bufs=2) as tstage_p,
        tc.tile_pool(name="psT", bufs=1, space="PSUM") as psT,
    ):
        ident0 = const_p.tile([128, 128], f32)
        ident = const_p.tile([128, 128], bf16)
        ones0 = const_p.tile([128, 128], f32)
        ones = const_p.tile([128, 128], bf16)
        cos_sb = const_p.tile([128, NTT, 64], f32)
        sin_sb = const_p.tile([128, NTT, 64], f32)
        v_sb = vsb_p.tile([128, NTT, NKV * HD], bf16)
        mask_sb = msk_p.tile([128, 384], bf16)
        preload: dict = {}

        # ================= PHASE 1: qkv = x @ Wqkv (+RoPE, +transposes) =====
        # W is streamed exactly once; x is re-streamed per column half; the
        # C-dim is split in two halves accumulated through SBUF (acc).
        xT_r = xT_d.rearrange("(cc p) t -> p cc t", p=128)  # [128, 32, 1024]
        with (
            tc.tile_pool(name="x", bufs=6) as x_p,
            tc.tile_pool(name="w", bufs=2) as w_p,
            tc.tile_pool(name="acc", bufs=1) as acc_p,
            tc.tile_pool(name="rtmp", bufs=3) as rtmp_p,
            tc.tile_pool(name="vstage", bufs=2) as vstage_p,
            tc.tile_pool(name="psA", bufs=3, space="PSUM") as psA,
        ):
            # second colhalf processes chalves in reverse so its first
            # segment reuses the x tiles already resident in SBUF
            segs = [(0, 0), (0, 1), (1, 1), (1, 0)]
            seg_tiles: dict = {}

            def emit_x(si_, q_, split=False):
                ch_, cf_ = segs[si_]
                xt = x_p.tile(
                    [128, 16, 256], bf16, tag="x", name=f"x{ch_}{cf_}{q_}"
                )
                base = 16 * cf_
                if split:
                    # fine-grained so the very first matmuls start early
                    for lo, hi in ((0, 4), (4, 8), (8, 16)):
                        nc.sync.dma_start(
                            out=xt[:, lo:hi, :],
                            in_=xT_r[:, base + lo : base + hi, ts(q_, 256)],
                        )
                else:
                    nc.sync.dma_start(
                        out=xt[:], in_=xT_r[:, base : base + 16, ts(q_, 256)]
                    )
                seg_tiles.setdefault(si_, {})[q_] = xt

            # startup: interleave the first x and W pieces so the very first
            # matmul's operands (x cc 0..3, W cc 0..1) land first
            xt00 = x_p.tile([128, 16, 256], bf16, tag="x", name="x000")
            seg_tiles[0] = {0: xt00}
            wt0 = w_p.tile([128, 16, 512], bf16, tag="w", name="w00")
            wsrc0 = wqkv_d[0, 0, 0].rearrange("cc p f -> p cc f")
            for (xlo, xhi), (wlo, whi) in zip(
                ((0, 4), (4, 8), (8, 16)), ((0, 2), (2, 8), (8, 16))
            ):
                nc.sync.dma_start(
                    out=xt00[:, xlo:xhi, :], in_=xT_r[:, xlo:xhi, ts(0, 256)]
                )
                nc.sync.dma_start(out=wt0[:, wlo:whi, :], in_=wsrc0[:, wlo:whi, :])
            # const setup after the first DMAs so those issue first
            make_identity(nc, ident0[:])
            nc.scalar.copy(out=ident[:], in_=ident0[:])
            nc.vector.memset(ones0[:], 1.0)
            nc.scalar.copy(out=ones[:], in_=ones0[:])
            acc = None
            pending = []
            for si_seg, (colhalf, chalf) in enumerate(segs):
                first = si_seg % 2 == 0  # first segment of this colhalf
                if first:
                    acc = acc_p.tile(
                        [128, NTT, 1536], f32, tag="acc", name=f"acc{colhalf}"
                    )
                if si_seg == 2:
                    seg_tiles[2] = seg_tiles[1]  # (1,1) reuses (0,1)'s x tiles
                xq = seg_tiles[si_seg]
                for j in range(3):
                    if si_seg == 0 and j == 0:
                        wt = wt0  # loaded during startup interleave
                    else:
                        wt = w_p.tile([128, 16, 512], bf16, tag="w")
                        wsrc = wqkv_d[colhalf, chalf, j].rearrange(
                            "cc p f -> p cc f"
                        )
                        nc.sync.dma_start(out=wt[:], in_=wsrc)
                    if j == 0:
                        if si_seg == 0:
                            emit_x(0, 1)
                        if si_seg != 2:
                            emit_x(si_seg, 2)
                            emit_x(si_seg, 3)
                        if si_seg == 0:
                            nc.sync.dma_start(
                                out=cos_sb[:],
                                in_=cos_d.rearrange("(tt p) j -> p tt j", p=128),
                            )
                            nc.sync.dma_start(
                                out=sin_sb[:],
                                in_=sin_d.rearrange("(tt p) j -> p tt j", p=128),
                            )
                    elif j == 1 and si_seg + 1 < len(segs) and si_seg + 1 != 2:
                        emit_x(si_seg + 1, 0)
                        emit_x(si_seg + 1, 1)
                    if si_seg == 2 and j == 0:
                        # colhalf 0 (k, v, q0..3) is final; flush its trailing
                        # transposes now so the preloads below can depend on
                        # them (si_seg==2 does no rope, which would otherwise
                        # delay the flush to si_seg==3).
                        for ppend in pending:
                            ppend()
                        pending = []
                    if si_seg == 2 and j == 1:
                        # preload the first attention inputs during si_seg 2,
                        # whose DMA queues are idle (it reuses si_seg 1's x
                        # tiles), so phase 2 starts with everything resident.
                        kt0 = kt_p.tile([128, T], bf16, tag="kt", name="kt0")
                        nc.sync.dma_start(out=kt0[:], in_=qkT_d[NQ])
                        preload["kt0"] = kt0
                        for hq_ in range(2):
                            qt0 = qt_p.tile(
                                [128, T], bf16, tag="qt", name=f"qt{hq_}"
                            )
                            nc.sync.dma_start(out=qt0[:], in_=qkT_d[hq_])
                            preload[f"qt{hq_}"] = qt0
                        nc.sync.dma_start(out=mask_sb[:], in_=masks_d[:])
                        vr = v_d.rearrange("(tt p) f -> p tt f", p=128)
                        nc.sync.dma_start(out=v_sb[:, 0:4, :], in_=vr[:, 0:4, :])
                        nc.sync.dma_start(out=v_sb[:, 4:8, :], in_=vr[:, 4:8, :])
                    for tt in range(NTT):
                        ps = psA.tile([128, 512], f32, tag="psA")
                        for cc in range(16):
                            nc.tensor.matmul(
                                ps[:],
                                xq[tt // 2][:, cc, ts(tt % 2, 128)],
                                wt[:, cc, :],
                                start=(cc == 0),
                                stop=(cc == 15),
                            )
                        dst = acc[:, tt, ts(j, 512)]
                        if first:
                            nc.scalar.copy(out=dst, in_=ps[:])
                            continue
                        nc.vector.tensor_add(dst, ps[:], dst)
                        if tt % 2 == 0:
                            continue
                        # (tt-1, tt) finalized -> rope batch + spill; the
                        # transposes of the previous batch are emitted now
                        # (one-batch software pipeline) so PE never waits on
                        # the rope chain.
                        slot = colhalf * 3 + j  # global 512-col chunk
                        t2p = tt - 1
                        if slot != 1:
                            a = acc[:, t2p : tt + 1, ts(j, 512)].rearrange(
                                "p t (h x j) -> p t h x j", x=2, j=64
                            )
                            cosb = (
                                cos_sb[:, t2p : tt + 1, :]
                                .unsqueeze(2)
                                .broadcast_to([128, 2, 4, 64])
                            )
                            sinb = (
                                sin_sb[:, t2p : tt + 1, :]
                                .unsqueeze(2)
                                .broadcast_to([128, 2, 4, 64])
                            )
                            rt = rope_p.tile([128, 2, 4, 2, 64], bf16, tag="rt")
                            t0 = rtmp_p.tile([128, 2, 4, 64], f32, tag="t0")
                            t1 = rtmp_p.tile([128, 2, 4, 64], f32, tag="t1")
                            t2 = rtmp_p.tile([128, 2, 4, 64], f32, tag="t2")
                            t3 = rtmp_p.tile([128, 2, 4, 64], f32, tag="t3")
                            nc.gpsimd.tensor_mul(t0[:], a[:, :, :, 0, :], cosb)
                            nc.gpsimd.tensor_mul(t1[:], a[:, :, :, 1, :], sinb)
                            nc.vector.tensor_sub(rt[:, :, :, 0, :], t0[:], t1[:])
                            nc.gpsimd.tensor_mul(t2[:], a[:, :, :, 1, :], cosb)
                            nc.gpsimd.tensor_mul(t3[:], a[:, :, :, 0, :], sinb)
                            nc.vector.tensor_add(rt[:, :, :, 1, :], t2[:], t3[:])
                            for ppend in pending:
                                ppend()
                            pending = []

                            def mk(rt_, slot_, t2p_):
                                def emit():
                                    h0 = SLOT_HEAD[slot_]
                                    # all 8 transposes of the batch into one
                                    # PSUM bank -> one wide copy + one DMA
                                    # with 512B lines
                                    pt = psT.tile([128, 4, 256], bf16, tag="psT")
                                    for ttl in range(2):
                                        for hh in range(4):
                                            nc.tensor.transpose(
                                                pt[:, hh, ts(ttl, 128)],
                                                rt_[:, ttl, hh].rearrange(
                                                    "p x j -> p (x j)"
                                                ),
                                                ident[:],
                                            )
                                    st = tstage_p.tile(
                                        [128, 4, 256], bf16, tag="ts"
                                    )
                                    nc.scalar.copy(out=st[:], in_=pt[:])
                                    nc.sync.dma_start(
                                        out=qkT_d[
                                            h0 : h0 + 4, :,
                                            128 * t2p_ : 128 * t2p_ + 256,
                                        ].rearrange("h p t -> p h t"),
                                        in_=st[:],
                                    )
                                return emit

                            pending.append(mk(rt, slot, t2p))
                        else:
                            vs = vstage_p.tile([128, 2, 512], bf16, tag="vs")
                            nc.scalar.copy(
                                out=vs[:], in_=acc[:, t2p : tt + 1, ts(j, 512)]
                            )
                            nc.sync.dma_start(
                                out=v_d[
                                    128 * t2p : 128 * (tt + 1), :
                                ].rearrange("(t p) f -> p t f", p=128),
                                in_=vs[:],
                            )
        # NOTE: the last rope batch's transposes stay in `pending`; they are
        # flushed after head 0's attention is emitted so the PE flows straight
        # from the last qkv matmul into score matmuls while the trailing rope
        # chain finishes on Pool/DVE.

        # ================= PHASE 2: attention ==============================
        with (
            tc.tile_pool(name="yt", bufs=NQ) as yt_p,
            tc.tile_pool(name="wp", bufs=2) as wp_p,
            tc.tile_pool(name="ostage", bufs=3) as ostage_p,
        ):
            yts = [
                yt_p.tile([128, T], bf16, tag="yt", name=f"yt{i}") for i in range(NQ)
            ]
            wps = {}

            with (
                tc.tile_pool(name="exp", bufs=5) as exp_p,
                tc.tile_pool(name="small", bufs=3) as small_p,
                tc.tile_pool(name="psS", bufs=3, space="PSUM") as psS,
                tc.tile_pool(name="psY", bufs=2, space="PSUM") as psY,
                tc.tile_pool(name="psD", bufs=2, space="PSUM") as psD,
            ):
                for g in range(NKV):
                    if g == 0:
                        kt = preload["kt0"]
                    else:
                        kt = kt_p.tile([128, T], bf16, tag="kt")
                        nc.sync.dma_start(out=kt[:], in_=qkT_d[NQ + g])
                    if g >= 1:
                        # prefetch the first Wproj block during attention,
                        # quartered to avoid head-of-line blocking qt loads
                        if g == 1:
                            wps[0] = wp_p.tile(
                                [128, 16, 512], bf16, tag="wp", name="wp0"
                            )
                        for qq in ([0, 1] if g == 1 else [2] if g == 2 else [3]):
                            nc.sync.dma_start(
                                out=wps[0][:, 4 * qq : 4 * (qq + 1), :],
                                in_=wproj_d[0, 4 * qq : 4 * (qq + 1)].rearrange(
                                    "y p f -> p y f"
                                ),
                            )
                    for r in range(REP):
                        hq = g * REP + r
                        # two-head lookahead on q loads
                        if hq + 2 < NQ and hq + 2 not in preload:
                            nxt = qt_p.tile(
                                [128, T], bf16, tag="qt", name=f"qt{hq + 2}"
                            )
                            nc.sync.dma_start(out=nxt[:], in_=qkT_d[hq + 2])
                            preload[hq + 2] = nxt
                        qt = preload.get(hq) or preload[f"qt{hq}"]

                        def emit_scores(chunk):
                            tq0 = 256 * chunk
                            npairs = chunk + 1
                            # scores + exp for every pair; the diagonal pair's
                            # second block is computed only for its valid
                            # upper t-half (compacted layout).
                            blocks = []  # (et, col0, width, si, py offset)
                            for pair in range(npairs):
                                si0 = 2 * pair
                                diag = pair == chunk
                                pss = psS.tile([128, 2, 256], f32, tag="psS")
                                flat = pss[:].rearrange("p a b -> p (a b)")
                                nc.tensor.matmul(
                                    pss[:, 0, :],
                                    kt[:, ts(si0, 128)],
                                    qt[:, tq0 : tq0 + 256],
                                    start=True,
                                    stop=True,
                                )
                                if not diag:
                                    nc.tensor.matmul(
                                        pss[:, 1, :],
                                        kt[:, ts(si0 + 1, 128)],
                                        qt[:, tq0 : tq0 + 256],
                                        start=True,
                                        stop=True,
                                    )
                                    et = exp_p.tile([128, 512], bf16, tag="exp")
                                    nc.scalar.activation(
                                        out=et[:], in_=flat, func=Exp, scale=SCALE
                                    )
                                    blocks.append((et, 0, 256, si0, 0))
                                    blocks.append((et, 256, 256, si0 + 1, 0))
                                else:
                                    nc.tensor.matmul(
                                        pss[:, 1, 0:128],
                                        kt[:, ts(si0 + 1, 128)],
                                        qt[:, tq0 + 128 : tq0 + 256],
                                        start=True,
                                        stop=True,
                                    )
                                    et = exp_p.tile([128, 384], bf16, tag="exp")
                                    nc.scalar.activation(
                                        out=et[:],
                                        in_=flat[:, 0:384],
                                        func=Exp,
                                        scale=SCALE,
                                    )
                                    nc.vector.tensor_mul(
                                        et[:], et[:], mask_sb[:]
                                    )
                                    blocks.append((et, 0, 256, si0, 0))
                                    blocks.append((et, 256, 128, si0 + 1, 128))
                            return blocks

                        def emit_av(chunk, blocks):
                            tq0 = 256 * chunk
                            ns = 2 * (chunk + 1)
                            # py = y accumulation; pd = softmax denominator
                            # (128-wide ones stationary puts the partition-sum
                            # on every output partition). Separate PSUM banks:
                            # interleaved accumulation groups must not share
                            # a bank.
                            py = psY.tile([128, 256], f32, tag="psY")
                            pd = psD.tile([128, 256], f32, tag="psD")
                            nb = len(blocks)
                            for b, (et, c0, w, si, off) in enumerate(blocks):
                                nc.tensor.matmul(
                                    py[:, off : off + w],
                                    v_sb[:, si, ts(g, 128)],
                                    et[:, c0 : c0 + w],
                                    start=(b == 0),
                                    stop=(b == nb - 1),
                                )
                                nc.tensor.matmul(
                                    pd[:, off : off + w],
                                    ones[:],
                                    et[:, c0 : c0 + w],
                                    start=(b == 0),
                                    stop=(b == nb - 1),
                                )
                            recip = small_p.tile([128, 256], f32, tag="recip")
                            nc.vector.reciprocal(out=recip[:], in_=pd[:])
                            nc.vector.tensor_mul(
                                yts[hq][:, tq0 : tq0 + 256], py[:], recip[:]
                            )

                        # software-pipeline: scores of chunk c+1 are emitted
                        # before the AV matmuls of chunk c so the in-order PE
                        # never waits on the Act exp chain
                        prev = None
                        for chunk in range(4):
                            blocks = emit_scores(chunk)
                            if prev is not None:
                                emit_av(*prev)
                            prev = (chunk, blocks)
                        emit_av(*prev)
                        if hq == 0:
                            for ppend in pending:
                                ppend()
                            pending = []

                # ============= PHASE 3: out = y @ Wproj (row shard) ============
                for ccol in range(8):
                    if ccol in wps:
                        wp = wps[ccol]
                    else:
                        wp = wp_p.tile([128, 16, 512], bf16, tag="wp")
                        nc.sync.dma_start(
                            out=wp[:, 0:8, :],
                            in_=wproj_d[ccol, 0:8].rearrange("y p f -> p y f"),
                        )
                        nc.sync.dma_start(
                            out=wp[:, 8:16, :],
                            in_=wproj_d[ccol, 8:16].rearrange("y p f -> p y f"),
                        )
                    for tt in range(NTT):
                        po = psS.tile([128, 512], f32, tag="psS")
                        for ycc in range(16):
                            nc.tensor.matmul(
                                po[:],
                                yts[ycc][:, ts(tt, 128)],
                                wp[:, ycc, :],
                                start=(ycc == 0),
                                stop=(ycc == 15),
                            )
                        ot = ostage_p.tile([128, 512], f32, tag="os")
                        if ccol == 7 and tt == NTT - 1:
                            # drain the last tile on two engines in parallel
                            nc.scalar.copy(out=ot[:, 0:256], in_=po[:, 0:256])
                            nc.vector.tensor_scalar_add(
                                ot[:, 256:512], po[:, 256:512], 0.0
                            )
                            nc.sync.dma_start(
                                out=out_d[ts(tt, 128), 512 * ccol : 512 * ccol + 256],
                                in_=ot[:, 0:256],
                            )
                            nc.sync.dma_start(
                                out=out_d[
                                    ts(tt, 128), 512 * ccol + 256 : 512 * ccol + 512
                                ],
                                in_=ot[:, 256:512],
                            )
                        else:
                            nc.scalar.copy(out=ot[:], in_=po[:])
                            nc.sync.dma_start(
                                out=out_d[ts(tt, 128), ts(ccol, 512)], in_=ot[:]
                            )

    nc.compile()
    return nc


def prep_inputs(x, Wqkv, Wproj, freqs_cos, freqs_sin):
    """Build the 8 per-core input maps (host-side shard + layout prep)."""
    x = np.asarray(x, np.float32)
    Wqkv = np.asarray(Wqkv, np.float32)
    Wproj = np.asarray(Wproj, np.float32)
    cos = np.ascontiguousarray(np.asarray(freqs_cos, np.float32))
    sin = np.ascontiguousarray(np.asarray(freqs_sin, np.float32))

    perm = np.concatenate([np.arange(0, HD, 2), np.arange(1, HD, 2)])
    p_ = np.arange(128)[:, None]
    masks = np.concatenate(
        [p_ <= np.arange(256)[None, :], p_ <= np.arange(128)[None, :]], axis=1
    ).astype(BF16)
    masks = np.ascontiguousarray(masks)

    in_maps = []
    for c in range(8):
        b, hh = divmod(c, 2)
        qcols = (hh * NQ * HD + (np.arange(NQ) * HD)[:, None] + perm[None, :]).ravel()
        kcols = (
            H * HD + hh * NKV * HD + (np.arange(NKV) * HD)[:, None] + perm[None, :]
        ).ravel()
        vcols = (
            (H + KV) * HD
            + hh * NKV * HD
            + (np.arange(NKV) * HD)[:, None]
            + np.arange(HD)[None, :]
        ).ravel()
        col_idx = np.concatenate([kcols, vcols, qcols])
        Wc = Wqkv[:, col_idx]  # [4096, 3072]
        wq = np.ascontiguousarray(
            Wc.reshape(2, 16, 128, 2, 3, 512).transpose(3, 0, 4, 1, 2, 5)
        ).astype(BF16)
        Wp = Wproj[hh * NQ * HD : (hh + 1) * NQ * HD, :]  # [2048, 4096]
        wp = np.ascontiguousarray(
            Wp.reshape(16, 128, 8, 512).transpose(2, 0, 1, 3)
        ).astype(BF16)
        xT = np.ascontiguousarray(x[b].T).astype(BF16)  # [4096, 1024]
        in_maps.append(
            {"xT": xT, "wqkv": wq, "wproj": wp, "cosn": cos, "sinn": sin,
             "masks": masks}
        )
    return in_maps


def _get_nc():
    if "nc" not in _CACHE:
        _CACHE["nc"] = _build_nc()
    return _CACHE["nc"]


def kernel(x, Wqkv, Wproj, freqs_cos, freqs_sin, mask=None):
    from concourse.bass_utils import run_bass_kernel_spmd

    nc = _get_nc()
    in_maps = prep_inputs(x, Wqkv, Wproj, freqs_cos, freqs_sin)
    res = run_bass_kernel_spmd(nc, in_maps, core_ids=list(range(8)))
    outs = [res.results[c]["out"] for c in range(8)]
    y = np.stack([outs[2 * b] + outs[2 * b + 1] for b in range(B)], axis=0)
    return y.astype(np.float32)


# revision 61
# speedup vs baseline: 1.1169x; 1.1169x over previous
"""Causal self-attention (GQA + RoPE) Trainium2 Bass kernel, 8-way sharded.

Sharding: core c -> batch b = c // 2, head-half hh = c % 2.
Each core computes the qkv projection, attention and output projection for
its batch and its 16 query heads / 4 kv heads (kv groups kept whole); the
output projection is a row-shard of Wproj, so the two cores of a batch
produce partial sums that the host adds.

Device-side layout tricks (host prepares):
  - all matmul operands are bf16 (halves HBM traffic; PE rate is identical
    to f32r for >=256-wide moving data, and strictly better below that).
  - x is fed pre-transposed (xT [C, T]) so the qkv matmul needs no on-device
    transpose of x.
  - per-core qkv columns are reordered to [k, v, q0..15] (k/v first so the
    attention inputs finalize early), with Wq/Wk columns de-interleaved per
    head (even rope pairs then odd) so RoPE is the rotate-half form with
    free-dim slices only.
  - scores are computed transposed (scoresT = k_tile^T-matmul) so the
    attention-weights matmul needs no transposes; the softmax denominator
    comes from a 128-wide all-ones stationary matmul (every output partition
    gets the partition-sum), so no extra broadcast pass is needed.
"""

import os

os.environ.setdefault("JAX_PLATFORMS", "axon")

import numpy as np
import ml_dtypes

BF16 = ml_dtypes.bfloat16

B, T, C = 4, 1024, 4096
H, KV, HD = 32, 8, 128
REP = H // KV  # 4

NQ = 16      # q heads per core
NKV = 4      # kv heads per core
COLS = (NQ + 2 * NKV) * HD   # 3072 local qkv cols: k0..3 v0..3 q0..15
NTT = T // 128               # 8 token tiles
SCALE = float(1.0 / np.sqrt(np.float32(HD)).astype(np.float32))

# 512-col slot -> first qkT head index (slot 1 is v, spilled untransposed)
SLOT_HEAD = {0: 16, 2: 0, 3: 4, 4: 8, 5: 12}

_CACHE: dict = {}


def _build_nc():
    import concourse.mybir as mybir
    import concourse.tile as tile
    from concourse import bacc
    from concourse.bass import ts
    from concourse.masks import make_identity

    f32 = mybir.dt.float32
    bf16 = mybir.dt.bfloat16
    Exp = mybir.ActivationFunctionType.Exp

    nc = bacc.Bacc(None, target_bir_lowering=False, debug=False)

    xT_d = nc.dram_tensor("xT", [C, T], bf16, kind="ExternalInput")
    # [colhalf, chalf, j(512-col chunk), cc(128-row chunk), 128, 512]
    wqkv_d = nc.dram_tensor("wqkv", [2, 2, 3, 16, 128, 512], bf16, kind="ExternalInput")
    # [ccol(512-col chunk), ycc(128-row chunk), 128, 512]
    wproj_d = nc.dram_tensor("wproj", [8, 16, 128, 512], bf16, kind="ExternalInput")
    cos_d = nc.dram_tensor("cosn", [T, 64], f32, kind="ExternalInput")
    sin_d = nc.dram_tensor("sinn", [T, 64], f32, kind="ExternalInput")
    # compact diagonal-pair mask: [p, 0:256] = (p <= t), [p, 256:384] =
    # (p <= t-128) for the half-width second diagonal block
    masks_d = nc.dram_tensor("masks", [128, 384], bf16, kind="ExternalInput")
    out_d = nc.dram_tensor("out", [T, C], f32, kind="ExternalOutput")
    # scratch: q/k transposed [head, hd=128, T] (0..15 q, 16..19 k); v natural
    qkT_d = nc.dram_tensor("qkT_scratch", [NQ + NKV, 128, T], bf16)
    v_d = nc.dram_tensor("v_scratch", [T, NKV * HD], bf16)

    with (
        tile.TileContext(nc) as tc,
        tc.tile_pool(name="const", bufs=1) as const_p,
        tc.tile_pool(name="vsb", bufs=1) as vsb_p,
        tc.tile_pool(name="msk", bufs=1) as msk_p,
        tc.tile_pool(name="qt", bufs=4) as qt_p,
        tc.tile_pool(name="kt", bufs=2) as kt_p,
        tc.tile_pool(name="rope", bufs=3) as rope_p,
        tc.tile_pool(name="tstage", bufs=2) as tstage_p,
        tc.tile_pool(name="psT", bufs=1, space="PSUM") as psT,
    ):
        ident0 = const_p.tile([128, 128], f32)
        ident = const_p.tile([128, 128], bf16)
        ones0 = const_p.tile([128, 128], f32)
        ones = const_p.tile([128, 128], bf16)
        cos_sb = const_p.tile([128, NTT, 64], f32)
        sin_sb = const_p.tile([128, NTT, 64], f32)
        v_sb = vsb_p.tile([128, NTT, NKV * HD], bf16)
        mask_sb = msk_p.tile([128, 384], bf16)
        preload: dict = {}

        # ================= PHASE 1: qkv = x @ Wqkv (+RoPE, +transposes) =====
        # W is streamed exactly once; x is re-streamed per column half; the
        # C-dim is split in two halves accumulated through SBUF (acc).
        xT_r = xT_d.rearrange("(cc p) t -> p cc t", p=128)  # [128, 32, 1024]
        with (
            tc.tile_pool(name="x", bufs=6) as x_p,
            tc.tile_pool(name="w", bufs=2) as w_p,
            tc.tile_pool(name="acc", bufs=1) as acc_p,
            tc.tile_pool(name="rtmp", bufs=3) as rtmp_p,
            tc.tile_pool(name="vstage", bufs=2) as vstage_p,
            tc.tile_pool(name="psA", bufs=3, space="PSUM") as psA,
        ):
            # second colhalf processes chalves in reverse so its first
            # segment reuses the x tiles already resident in SBUF
            segs = [(0, 0), (0, 1), (1, 1), (1, 0)]
            seg_tiles: dict = {}

            def emit_x(si_, q_, split=False):
                ch_, cf_ = segs[si_]
                xt = x_p.tile(
                    [128, 16, 256], bf16, tag="x", name=f"x{ch_}{cf_}{q_}"
                )
                base = 16 * cf_
                if split:
                    # fine-grained so the very first matmuls start early
                    for lo, hi in ((0, 4), (4, 8), (8, 16)):
                        nc.sync.dma_start(
                            out=xt[:, lo:hi, :],
                            in_=xT_r[:, base + lo : base + hi, ts(q_, 256)],
                        )
                else:
                    nc.sync.dma_start(
                        out=xt[:], in_=xT_r[:, base : base + 16, ts(q_, 256)]
                    )
                seg_tiles.setdefault(si_, {})[q_] = xt

            # startup: interleave the first x and W pieces so the very first
            # matmul's operands (x cc 0..3, W cc 0..1) land first
            xt00 = x_p.tile([128, 16, 256], bf16, tag="x", name="x000")
            seg_tiles[0] = {0: xt00}
            wt0 = w_p.tile([128, 16, 512], bf16, tag="w", name="w00")
            wsrc0 = wqkv_d[0, 0, 0].rearrange("cc p f -> p cc f")
            for (xlo, xhi), (wlo, whi) in zip(
                ((0, 4), (4, 8), (8, 16)), ((0, 2), (2, 8), (8, 16))
            ):
                nc.sync.dma_start(
                    out=xt00[:, xlo:xhi, :], in_=xT_r[:, xlo:xhi, ts(0, 256)]
                )
                nc.sync.dma_start(out=wt0[:, wlo:whi, :], in_=wsrc0[:, wlo:whi, :])
            # const setup after the first DMAs so those issue first
            make_identity(nc, ident0[:])
            nc.scalar.copy(out=ident[:], in_=ident0[:])
            nc.vector.memset(ones0[:], 1.0)
            nc.scalar.copy(out=ones[:], in_=ones0[:])
            acc = None
            pending = []
            for si_seg, (colhalf, chalf) in enumerate(segs):
                first = si_seg % 2 == 0  # first segment of this colhalf
                if first:
                    acc = acc_p.tile(
                        [128, NTT, 1536], f32, tag="acc", name=f"acc{colhalf}"
                    )
                if si_seg == 2:
                    seg_tiles[2] = seg_tiles[1]  # (1,1) reuses (0,1)'s x tiles
                xq = seg_tiles[si_seg]
                for j in range(3):
                    if si_seg == 0 and j == 0:
                        wt = wt0  # loaded during startup interleave
                    else:
                        wt = w_p.tile([128, 16, 512], bf16, tag="w")
                        wsrc = wqkv_d[colhalf, chalf, j].rearrange(
                            "cc p f -> p cc f"
                        )
                        nc.sync.dma_start(out=wt[:], in_=wsrc)
                    if j == 0:
                        if si_seg == 0:
                            emit_x(0, 1)
                        if si_seg != 2:
                            emit_x(si_seg, 2)
                            emit_x(si_seg, 3)
                        if si_seg == 0:
                            nc.sync.dma_start(
                                out=cos_sb[:],
                                in_=cos_d.rearrange("(tt p) j -> p tt j", p=128),
                            )
                            nc.sync.dma_start(
                                out=sin_sb[:],
                                in_=sin_d.rearrange("(tt p) j -> p tt j", p=128),
                            )
                    elif j == 1 and si_seg + 1 < len(segs) and si_seg + 1 != 2:
                        emit_x(si_seg + 1, 0)
                        emit_x(si_seg + 1, 1)
                    if si_seg == 2 and j == 0:
                        # colhalf 0 (k, v, q0..3) is final; flush its trailing
                        # transposes now so the preloads below can depend on
                        # them (si_seg==2 does no rope, which would otherwise
                        # delay the flush to si_seg==3).
                        for ppend in pending:
                            ppend()
                        pending = []
                    if si_seg == 2 and j == 1:
                        # preload the first attention inputs during si_seg 2,
                        # whose DMA queues are idle (it reuses si_seg 1's x
                        # tiles), so phase 2 starts with everything resident.
                        kt0 = kt_p.tile([128, T], bf16, tag="kt", name="kt0")
                        nc.sync.dma_start(out=kt0[:], in_=qkT_d[NQ])
                        preload["kt0"] = kt0
                        for hq_ in range(2):
                            qt0 = qt_p.tile(
                                [128, T], bf16, tag="qt", name=f"qt{hq_}"
                            )
                            nc.sync.dma_start(out=qt0[:], in_=qkT_d[hq_])
                            preload[f"qt{hq_}"] = qt0
                        nc.sync.dma_start(out=mask_sb[:], in_=masks_d[:])
                        vr = v_d.rearrange("(tt p) f -> p tt f", p=128)
                        nc.sync.dma_start(out=v_sb[:, 0:4, :], in_=vr[:, 0:4, :])
                        nc.sync.dma_start(out=v_sb[:, 4:8, :], in_=vr[:, 4:8, :])
                    for tt in range(NTT):
                        ps = psA.tile([128, 512], f32, tag="psA")
                        for cc in range(16):
                            nc.tensor.matmul(
                                ps[:],
                                xq[tt // 2][:, cc, ts(tt % 2, 128)],
                                wt[:, cc, :],
                                start=(cc == 0),
                                stop=(cc == 15),
                            )
                        dst = acc[:, tt, ts(j, 512)]
                        if first:
                            nc.scalar.copy(out=dst, in_=ps[:])
                            continue
                        nc.vector.tensor_add(dst, ps[:], dst)
                        if tt % 2 == 0:
                            continue
                        # (tt-1, tt) finalized -> rope batch + spill; the
                        # transposes of the previous batch are emitted now
                        # (one-batch software pipeline) so PE never waits on
                        # the rope chain.
                        slot = colhalf * 3 + j  # global 512-col chunk
                        t2p = tt - 1
                        if slot != 1:
                            a = acc[:, t2p : tt + 1, ts(j, 512)].rearrange(
                                "p t (h x j) -> p t h x j", x=2, j=64
                            )
                            cosb = (
                                cos_sb[:, t2p : tt + 1, :]
                                .unsqueeze(2)
                                .broadcast_to([128, 2, 4, 64])
                            )
                            sinb = (
                                sin_sb[:, t2p : tt + 1, :]
                                .unsqueeze(2)
                                .broadcast_to([128, 2, 4, 64])
                            )
                            rt = rope_p.tile([128, 2, 4, 2, 64], bf16, tag="rt")
                            t0 = rtmp_p.tile([128, 2, 4, 64], f32, tag="t0")
                            t1 = rtmp_p.tile([128, 2, 4, 64], f32, tag="t1")
                            t2 = rtmp_p.tile([128, 2, 4, 64], f32, tag="t2")
                            t3 = rtmp_p.tile([128, 2, 4, 64], f32, tag="t3")
                            nc.gpsimd.tensor_mul(t0[:], a[:, :, :, 0, :], cosb)
                            nc.gpsimd.tensor_mul(t1[:], a[:, :, :, 1, :], sinb)
                            nc.vector.tensor_sub(rt[:, :, :, 0, :], t0[:], t1[:])
                            nc.gpsimd.tensor_mul(t2[:], a[:, :, :, 1, :], cosb)
                            nc.gpsimd.tensor_mul(t3[:], a[:, :, :, 0, :], sinb)
                            nc.vector.tensor_add(rt[:, :, :, 1, :], t2[:], t3[:])
                            for ppend in pending:
                                ppend()
                            pending = []

                            def mk(rt_, slot_, t2p_):
                                def emit():
                                    h0 = SLOT_HEAD[slot_]
                                    # all 8 transposes of the batch into one
                                    # PSUM bank -> one wide copy + one DMA
                                    # with 512B lines
                                    pt = psT.tile([128, 4, 256], bf16, tag="psT")
                                    for ttl in range(2):
                                        for hh in range(4):
                                            nc.tensor.transpose(
                                                pt[:, hh, ts(ttl, 128)],
                                                rt_[:, ttl, hh].rearrange(
                                                    "p x j -> p (x j)"
                                                ),
                                                ident[:],
                                            )
                                    st = tstage_p.tile(
                                        [128, 4, 256], bf16, tag="ts"
                                    )
                                    nc.scalar.copy(out=st[:], in_=pt[:])
                                    nc.sync.dma_start(
                                        out=qkT_d[
                                            h0 : h0 + 4, :,
                                            128 * t2p_ : 128 * t2p_ + 256,
                                        ].rearrange("h p t -> p h t"),
                                        in_=st[:],
                                    )
                                return emit

                            pending.append(mk(rt, slot, t2p))
                        else:
                            vs = vstage_p.tile([128, 2, 512], bf16, tag="vs")
                            nc.scalar.copy(
                                out=vs[:], in_=acc[:, t2p : tt + 1, ts(j, 512)]
                            )
                            nc.sync.dma_start(
                                out=v_d[
                                    128 * t2p : 128 * (tt + 1), :
                                ].rearrange("(t p) f -> p t f", p=128),
                                in_=vs[:],
                            )
        # NOTE: the last rope batch's transposes stay in `pending`; they are
        # flushed after head 0's attention is emitted so the PE flows straight
        # from the last qkv matmul into score matmuls while the trailing rope
        # chain finishes on Pool/DVE.

        # ================= PHASE 2: attention ==============================
        with (
            tc.tile_pool(name="yt", bufs=NQ) as yt_p,
            tc.tile_pool(name="wp", bufs=2) as wp_p,
            tc.tile_pool(name="ostage", bufs=3) as ostage_p,
        ):
            yts = [
                yt_p.tile([128, T], bf16, tag="yt", name=f"yt{i}") for i in range(NQ)
            ]
            wps = {}

            with (
                tc.tile_pool(name="exp", bufs=5) as exp_p,
                tc.tile_pool(name="small", bufs=3) as small_p,
                tc.tile_pool(name="psS", bufs=3, space="PSUM") as psS,
                tc.tile_pool(name="psY", bufs=2, space="PSUM") as psY,
                tc.tile_pool(name="psD", bufs=2, space="PSUM") as psD,
            ):
                for g in range(NKV):
                    if g == 0:
                        kt = preload["kt0"]
                    else:
                        kt = kt_p.tile([128, T], bf16, tag="kt")
                        nc.sync.dma_start(out=kt[:], in_=qkT_d[NQ + g])
                    if g >= 1:
                        # prefetch the first Wproj block during attention,
                        # quartered to avoid head-of-line blocking qt loads
                        if g == 1:
                            wps[0] = wp_p.tile(
                                [128, 16, 512], bf16, tag="wp", name="wp0"
                            )
                        for qq in ([0, 1] if g == 1 else [2] if g == 2 else [3]):
                            nc.sync.dma_start(
                                out=wps[0][:, 4 * qq : 4 * (qq + 1), :],
                                in_=wproj_d[0, 4 * qq : 4 * (qq + 1)].rearrange(
                                    "y p f -> p y f"
                                ),
                            )
                    for r in range(REP):
                        hq = g * REP + r
                        # two-head lookahead on q loads
                        if hq + 2 < NQ and hq + 2 not in preload:
                            nxt = qt_p.tile(
                                [128, T], bf16, tag="qt", name=f"qt{hq + 2}"
                            )
                            nc.sync.dma_start(out=nxt[:], in_=qkT_d[hq + 2])
                            preload[hq + 2] = nxt
                        qt = preload.get(hq) or preload[f"qt{hq}"]

                        def emit_scores(chunk):
                            tq0 = 256 * chunk
                            npairs = chunk + 1
                            # scores + exp for every pair; the diagonal pair's
                            # second block is computed only for its valid
                            # upper t-half (compacted layout).
                            blocks = []  # (et, col0, width, si, py offset)
                            for pair in range(npairs):
                                si0 = 2 * pair
                                diag = pair == chunk
                                pss = psS.tile([128, 2, 256], f32, tag="psS")
                                flat = pss[:].rearrange("p a b -> p (a b)")
                                nc.tensor.matmul(
                                    pss[:, 0, :],
                                    kt[:, ts(si0, 128)],
                                    qt[:, tq0 : tq0 + 256],
                                    start=True,
                                    stop=True,
                                )
                                if not diag:
                                    nc.tensor.matmul(
                                        pss[:, 1, :],
                                        kt[:, ts(si0 + 1, 128)],
                                        qt[:, tq0 : tq0 + 256],
                                        start=True,
                                        stop=True,
                                    )
                                    et = exp_p.tile([128, 512], bf16, tag="exp")
                                    nc.scalar.activation(
                                        out=et[:], in_=flat, func=Exp, scale=SCALE
                                    )
                                    blocks.append((et, 0, 256, si0, 0))
                                    blocks.append((et, 256, 256, si0 + 1, 0))
                                else:
                                    nc.tensor.matmul(
                                        pss[:, 1, 0:128],
                                        kt[:, ts(si0 + 1, 128)],
                                        qt[:, tq0 + 128 : tq0 + 256],
                                        start=True,
                                        stop=True,
                                    )
                                    et = exp_p.tile([128, 384], bf16, tag="exp")
                                    nc.scalar.activation(
                                        out=et[:],
                                        in_=flat[:, 0:384],
                                        func=Exp,
                                        scale=SCALE,
                                    )
                                    nc.vector.tensor_mul(
                                        et[:], et[:], mask_sb[:]
                                    )
                                    blocks.append((et, 0, 256, si0, 0))
                                    blocks.append((et, 256, 128, si0 + 1, 128))
                            return blocks

                        def emit_av(chunk, blocks):
                            tq0 = 256 * chunk
                            ns = 2 * (chunk + 1)
                            # py = y accumulation; pd = softmax denominator
                            # (128-wide ones stationary puts the partition-sum
                            # on every output partition). Separate PSUM banks:
                            # interleaved accumulation groups must not share
                            # a bank.
                            py = psY.tile([128, 256], f32, tag="psY")
                            pd = psD.tile([128, 256], f32, tag="psD")
                            nb = len(blocks)
                            for b, (et, c0, w, si, off) in enumerate(blocks):
                                nc.tensor.matmul(
                                    py[:, off : off + w],
                                    v_sb[:, si, ts(g, 128)],
                                    et[:, c0 : c0 + w],
                                    start=(b == 0),
                                    stop=(b == nb - 1),
                                )
                                nc.tensor.matmul(
                                    pd[:, off : off + w],
                                    ones[:],
                                    et[:, c0 : c0 + w],
                                    start=(b == 0),
                                    stop=(b == nb - 1),
                                )
                            recip = small_p.tile([128, 256], f32, tag="recip")
                            nc.vector.reciprocal(out=recip[:], in_=pd[:])
                            nc.vector.tensor_mul(
                                yts[hq][:, tq0 : tq0 + 256], py[:], recip[:]
                            )

                        # software-pipeline: scores of chunk c+1 are emitted
                        # before the AV matmuls of chunk c so the in-order PE
                        # never waits on the Act exp chain
                        prev = None
                        for chunk in range(4):
                            blocks = emit_scores(chunk)
                            if prev is not None:
                                emit_av(*prev)
                            prev = (chunk, blocks)
                        emit_av(*prev)
                        if hq == 0:
                            for ppend in pending:
                                ppend()
                            pending = []

                # ============= PHASE 3: out = y @ Wproj (row shard) ============
                for ccol in range(8):
                    if ccol in wps:
                        wp = wps[ccol]
                    else:
                        wp = wp_p.tile([128, 16, 512], bf16, tag="wp")
                        nc.sync.dma_start(
                            out=wp[:, 0:8, :],
                            in_=wproj_d[ccol, 0:8].rearrange("y p f -> p y f"),
                        )
                        nc.sync.dma_start(
                            out=wp[:, 8:16, :],
                            in_=wproj_d[ccol, 8:16].rearrange("y p f -> p y f"),
                        )
                    for tt in range(NTT):
                        po = psS.tile([128, 512], f32, tag="psS")
                        for ycc in range(16):
                            nc.tensor.matmul(
                                po[:],
                                yts[ycc][:, ts(tt, 128)],
                                wp[:, ycc, :],
                                start=(ycc == 0),
                                stop=(ycc == 15),
                            )
                        ot = ostage_p.tile([128, 512], f32, tag="os")
                        if ccol == 7 and tt == NTT - 1:
                            # drain the last tile on two engines in parallel
                            nc.scalar.copy(out=ot[:, 0:256], in_=po[:, 0:256])
                            nc.vector.tensor_scalar_add(
                                ot[:, 256:512], po[:, 256:512], 0.0
                            )
                            nc.sync.dma_start(
                                out=out_d[ts(tt, 128), 512 * ccol : 512 * ccol + 256],
                                in_=ot[:, 0:256],
                            )
                            nc.sync.dma_start(
                                out=out_d[
                                    ts(tt, 128), 512 * ccol + 256 : 512 * ccol + 512
                                ],
                                in_=ot[:, 256:512],
                            )
                        else:
                            nc.scalar.copy(out=ot[:], in_=po[:])
                            nc.sync.dma_start(
                                out=out_d[ts(tt, 128), ts(ccol, 512)], in_=ot[:]
                            )

    nc.compile()
    return nc


def prep_inputs(x, Wqkv, Wproj, freqs_cos, freqs_sin):
    """Build the 8 per-core input maps (host-side shard + layout prep)."""
    x = np.asarray(x, np.float32)
    Wqkv = np.asarray(Wqkv, np.float32)
    Wproj = np.asarray(Wproj, np.float32)
    cos = np.ascontiguousarray(np.asarray(freqs_cos, np.float32))
    sin = np.ascontiguousarray(np.asarray(freqs_sin, np.float32))

    perm = np.concatenate([np.arange(0, HD, 2), np.arange(1, HD, 2)])
    p_ = np.arange(128)[:, None]
    masks = np.concatenate(
        [p_ <= np.arange(256)[None, :], p_ <= np.arange(128)[None, :]], axis=1
    ).astype(BF16)
    masks = np.ascontiguousarray(masks)

    in_maps = []
    for c in range(8):
        b, hh = divmod(c, 2)
        qcols = (hh * NQ * HD + (np.arange(NQ) * HD)[:, None] + perm[None, :]).ravel()
        kcols = (
            H * HD + hh * NKV * HD + (np.arange(NKV) * HD)[:, None] + perm[None, :]
        ).ravel()
        vcols = (
            (H + KV) * HD
            + hh * NKV * HD
            + (np.arange(NKV) * HD)[:, None]
            + np.arange(HD)[None, :]
        ).ravel()
        col_idx = np.concatenate([kcols, vcols, qcols])
        Wc = Wqkv[:, col_idx]  # [4096, 3072]
        wq = np.ascontiguousarray(
            Wc.reshape(2, 16, 128, 2, 3, 512).transpose(3, 0, 4, 1, 2, 5)
        ).astype(BF16)
        Wp = Wproj[hh * NQ * HD : (hh + 1) * NQ * HD, :]  # [2048, 4096]
        wp = np.ascontiguousarray(
            Wp.reshape(16, 128, 8, 512).transpose(2, 0, 1, 3)
        ).astype(BF16)
        xT = np.ascontiguousarray(x[b].T).astype(BF16)  # [4096, 1024]
        in_maps.append(
            {"xT": xT, "wqkv": wq, "wproj": wp, "cosn": cos, "sinn": sin,
             "masks": masks}
        )
    return in_maps


def _get_nc():
    if "nc" not in _CACHE:
        _CACHE["nc"] = _build_nc()
    return _CACHE["nc"]


def kernel(x, Wqkv, Wproj, freqs_cos, freqs_sin, mask=None):
    from concourse.bass_utils import run_bass_kernel_spmd

    nc = _get_nc()
    in_maps = prep_inputs(x, Wqkv, Wproj, freqs_cos, freqs_sin)
    res = run_bass_kernel_spmd(nc, in_maps, core_ids=list(range(8)))
    outs = [res.results[c]["out"] for c in range(8)]
    y = np.stack([outs[2 * b] + outs[2 * b + 1] for b in range(B)], axis=0)
    return y.astype(np.float32)


# revision 63
# speedup vs baseline: 1.6700x; 1.4952x over previous
"""Causal self-attention (GQA + RoPE) Trainium2 Bass kernel, 8-way sharded.

Sharding: core c -> batch b = c // 2, head-half hh = c % 2.
Each core computes the qkv projection, attention and output projection for
its batch and its 16 query heads / 4 kv heads (kv groups kept whole); the
output projection is a row-shard of Wproj, so the two cores of a batch
produce partial sums that the host adds.

Device-side layout tricks (host prepares):
  - all matmul operands are bf16 (halves HBM traffic; PE rate is identical
    to f32r for >=256-wide moving data, and strictly better below that).
  - x is fed pre-transposed (xT [C, T]) so the qkv matmul needs no on-device
    transpose of x.
  - per-core qkv columns are reordered to [k, v, q0..15] (k/v first so the
    attention inputs finalize early), with Wq/Wk columns de-interleaved per
    head (even rope pairs then odd) so RoPE is the rotate-half form with
    free-dim slices only.
  - scores are computed transposed (scoresT = k_tile^T-matmul) so the
    attention-weights matmul needs no transposes; the softmax denominator
    comes from a 128-wide all-ones stationary matmul (every output partition
    gets the partition-sum), so no extra broadcast pass is needed.
"""

import os

os.environ.setdefault("JAX_PLATFORMS", "axon")

import numpy as np
import ml_dtypes

BF16 = ml_dtypes.bfloat16

B, T, C = 4, 1024, 4096
H, KV, HD = 32, 8, 128
REP = H // KV  # 4

NQ = 16      # q heads per core
NKV = 4      # kv heads per core
COLS = (NQ + 2 * NKV) * HD   # 3072 local qkv cols: k0..3 v0..3 q0..15
NTT = T // 128               # 8 token tiles
SCALE = float(1.0 / np.sqrt(np.float32(HD)).astype(np.float32))

# 512-col slot -> first qkT head index (slot 1 is v, spilled untransposed)
SLOT_HEAD = {0: 16, 2: 0, 3: 4, 4: 8, 5: 12}

_CACHE: dict = {}


def _build_nc():
    import concourse.mybir as mybir
    import concourse.tile as tile
    from concourse import bacc
    from concourse.bass import ts
    from concourse.masks import make_identity

    f32 = mybir.dt.float32
    bf16 = mybir.dt.bfloat16
    Exp = mybir.ActivationFunctionType.Exp

    nc = bacc.Bacc(None, target_bir_lowering=False, debug=False)

    xT_d = nc.dram_tensor("xT", [C, T], bf16, kind="ExternalInput")
    # [colhalf, chalf, j(512-col chunk), cc(128-row chunk), 128, 512]
    wqkv_d = nc.dram_tensor("wqkv", [2, 2, 3, 16, 128, 512], bf16, kind="ExternalInput")
    # [ccol(512-col chunk), ycc(128-row chunk), 128, 512]
    wproj_d = nc.dram_tensor("wproj", [8, 16, 128, 512], bf16, kind="ExternalInput")
    cos_d = nc.dram_tensor("cosn", [T, 64], f32, kind="ExternalInput")
    sin_d = nc.dram_tensor("sinn", [T, 64], f32, kind="ExternalInput")
    # compact diagonal-pair mask: [p, 0:256] = (p <= t), [p, 256:384] =
    # (p <= t-128) for the half-width second diagonal block
    masks_d = nc.dram_tensor("masks", [128, 384], bf16, kind="ExternalInput")
    out_d = nc.dram_tensor("out", [T, C], f32, kind="ExternalOutput")
    # scratch: q/k transposed [head, hd=128, T] (0..15 q, 16..19 k); v natural
    qkT_d = nc.dram_tensor("qkT_scratch", [NQ + NKV, 128, T], bf16)
    v_d = nc.dram_tensor("v_scratch", [T, NKV * HD], bf16)

    with (
        tile.TileContext(nc) as tc,
        tc.tile_pool(name="const", bufs=1) as const_p,
        tc.tile_pool(name="vsb", bufs=1) as vsb_p,
        tc.tile_pool(name="msk", bufs=1) as msk_p,
        tc.tile_pool(name="qt", bufs=4) as qt_p,
        tc.tile_pool(name="kt", bufs=2) as kt_p,
        tc.tile_pool(name="rope", bufs=3) as rope_p,
        tc.tile_pool(name="tstage", bufs=2) as tstage_p,
        tc.tile_pool(name="psT", bufs=1, space="PSUM") as psT,
    ):
        ident0 = const_p.tile([128, 128], f32)
        ident = const_p.tile([128, 128], bf16)
        ones0 = const_p.tile([128, 128], f32)
        ones = const_p.tile([128, 128], bf16)
        cos_sb = const_p.tile([128, NTT, 64], f32)
        sin_sb = const_p.tile([128, NTT, 64], f32)
        v_sb = vsb_p.tile([128, NTT, NKV * HD], bf16)
        mask_sb = msk_p.tile([128, 384], bf16)
        preload: dict = {}

        # ================= PHASE 1: qkv = x @ Wqkv (+RoPE, +transposes) =====
        # W is streamed exactly once; x is re-streamed per column half; the
        # C-dim is split in two halves accumulated through SBUF (acc).
        xT_r = xT_d.rearrange("(cc p) t -> p cc t", p=128)  # [128, 32, 1024]
        with (
            tc.tile_pool(name="x", bufs=6) as x_p,
            tc.tile_pool(name="w", bufs=2) as w_p,
            tc.tile_pool(name="acc", bufs=1) as acc_p,
            tc.tile_pool(name="rtmp", bufs=3) as rtmp_p,
            tc.tile_pool(name="vstage", bufs=2) as vstage_p,
            tc.tile_pool(name="psA", bufs=3, space="PSUM") as psA,
        ):
            # second colhalf processes chalves in reverse so its first
            # segment reuses the x tiles already resident in SBUF
            segs = [(0, 0), (0, 1), (1, 1), (1, 0)]
            seg_tiles: dict = {}

            def emit_x(si_, q_, split=False):
                ch_, cf_ = segs[si_]
                xt = x_p.tile(
                    [128, 16, 256], bf16, tag="x", name=f"x{ch_}{cf_}{q_}"
                )
                base = 16 * cf_
                if split:
                    # fine-grained so the very first matmuls start early
                    for lo, hi in ((0, 4), (4, 8), (8, 16)):
                        nc.sync.dma_start(
                            out=xt[:, lo:hi, :],
                            in_=xT_r[:, base + lo : base + hi, ts(q_, 256)],
                        )
                else:
                    nc.sync.dma_start(
                        out=xt[:], in_=xT_r[:, base : base + 16, ts(q_, 256)]
                    )
                seg_tiles.setdefault(si_, {})[q_] = xt

            # startup: interleave the first x and W pieces so the very first
            # matmul's operands (x cc 0..3, W cc 0..1) land first
            xt00 = x_p.tile([128, 16, 256], bf16, tag="x", name="x000")
            seg_tiles[0] = {0: xt00}
            wt0 = w_p.tile([128, 16, 512], bf16, tag="w", name="w00")
            wsrc0 = wqkv_d[0, 0, 0].rearrange("cc p f -> p cc f")
            for (xlo, xhi), (wlo, whi) in zip(
                ((0, 4), (4, 8), (8, 16)), ((0, 2), (2, 8), (8, 16))
            ):
                nc.sync.dma_start(
                    out=xt00[:, xlo:xhi, :], in_=xT_r[:, xlo:xhi, ts(0, 256)]
                )
                nc.sync.dma_start(out=wt0[:, wlo:whi, :], in_=wsrc0[:, wlo:whi, :])
            # const setup after the first DMAs so those issue first
            make_identity(nc, ident0[:])
            nc.scalar.copy(out=ident[:], in_=ident0[:])
            nc.vector.memset(ones0[:], 1.0)
            nc.scalar.copy(out=ones[:], in_=ones0[:])
            acc = None
            pending = []
            for si_seg, (colhalf, chalf) in enumerate(segs):
                first = si_seg % 2 == 0  # first segment of this colhalf
                if first:
                    acc = acc_p.tile(
                        [128, NTT, 1536], f32, tag="acc", name=f"acc{colhalf}"
                    )
                if si_seg == 2:
                    seg_tiles[2] = seg_tiles[1]  # (1,1) reuses (0,1)'s x tiles
                xq = seg_tiles[si_seg]
                for j in range(3):
                    if si_seg == 0 and j == 0:
                        wt = wt0  # loaded during startup interleave
                    else:
                        wt = w_p.tile([128, 16, 512], bf16, tag="w")
                        wsrc = wqkv_d[colhalf, chalf, j].rearrange(
                            "cc p f -> p cc f"
                        )
                        nc.sync.dma_start(out=wt[:], in_=wsrc)
                    if j == 0:
                        if si_seg == 0:
                            emit_x(0, 1)
                        if si_seg != 2:
                            emit_x(si_seg, 2)
                            emit_x(si_seg, 3)
                        if si_seg == 0:
                            nc.sync.dma_start(
                                out=cos_sb[:],
                                in_=cos_d.rearrange("(tt p) j -> p tt j", p=128),
                            )
                            nc.sync.dma_start(
                                out=sin_sb[:],
                                in_=sin_d.rearrange("(tt p) j -> p tt j", p=128),
                            )
                    elif j == 1 and si_seg + 1 < len(segs) and si_seg + 1 != 2:
                        emit_x(si_seg + 1, 0)
                        emit_x(si_seg + 1, 1)
                    if si_seg == 2 and j == 0:
                        # colhalf 0 (k, v, q0..3) is final; flush its trailing
                        # transposes now so the preloads below can depend on
                        # them (si_seg==2 does no rope, which would otherwise
                        # delay the flush to si_seg==3).
                        for ppend in pending:
                            ppend()
                        pending = []
                    if si_seg == 2 and j == 1:
                        # preload the first attention inputs during si_seg 2,
                        # whose DMA queues are idle (it reuses si_seg 1's x
                        # tiles), so phase 2 starts with everything resident.
                        kt0 = kt_p.tile([128, T], bf16, tag="kt", name="kt0")
                        nc.sync.dma_start(out=kt0[:], in_=qkT_d[NQ])
                        preload["kt0"] = kt0
                        for hq_ in range(2):
                            qt0 = qt_p.tile(
                                [128, T], bf16, tag="qt", name=f"qt{hq_}"
                            )
                            nc.sync.dma_start(out=qt0[:], in_=qkT_d[hq_])
                            preload[f"qt{hq_}"] = qt0
                        nc.sync.dma_start(out=mask_sb[:], in_=masks_d[:])
                        vr = v_d.rearrange("(tt p) f -> p tt f", p=128)
                        nc.sync.dma_start(out=v_sb[:, 0:4, :], in_=vr[:, 0:4, :])
                        nc.sync.dma_start(out=v_sb[:, 4:8, :], in_=vr[:, 4:8, :])
                    for tt in range(NTT):
                        ps = psA.tile([128, 512], f32, tag="psA")
                        for cc in range(16):
                            nc.tensor.matmul(
                                ps[:],
                                xq[tt // 2][:, cc, ts(tt % 2, 128)],
                                wt[:, cc, :],
                                start=(cc == 0),
                                stop=(cc == 15),
                            )
                        dst = acc[:, tt, ts(j, 512)]
                        if first:
                            nc.scalar.copy(out=dst, in_=ps[:])
                            continue
                        nc.vector.tensor_add(dst, ps[:], dst)
                        if tt % 2 == 0:
                            continue
                        # (tt-1, tt) finalized -> rope batch + spill; the
                        # transposes of the previous batch are emitted now
                        # (one-batch software pipeline) so PE never waits on
                        # the rope chain.
                        slot = colhalf * 3 + j  # global 512-col chunk
                        t2p = tt - 1
                        if slot != 1:
                            a = acc[:, t2p : tt + 1, ts(j, 512)].rearrange(
                                "p t (h x j) -> p t h x j", x=2, j=64
                            )
                            cosb = (
                                cos_sb[:, t2p : tt + 1, :]
                                .unsqueeze(2)
                                .broadcast_to([128, 2, 4, 64])
                            )
                            sinb = (
                                sin_sb[:, t2p : tt + 1, :]
                                .unsqueeze(2)
                                .broadcast_to([128, 2, 4, 64])
                            )
                            rt = rope_p.tile([128, 2, 4, 2, 64], bf16, tag="rt")
                            t0 = rtmp_p.tile([128, 2, 4, 64], f32, tag="t0")
                            t1 = rtmp_p.tile([128, 2, 4, 64], f32, tag="t1")
                            t2 = rtmp_p.tile([128, 2, 4, 64], f32, tag="t2")
                            t3 = rtmp_p.tile([128, 2, 4, 64], f32, tag="t3")
                            nc.gpsimd.tensor_mul(t0[:], a[:, :, :, 0, :], cosb)
                            nc.gpsimd.tensor_mul(t1[:], a[:, :, :, 1, :], sinb)
                            nc.vector.tensor_sub(rt[:, :, :, 0, :], t0[:], t1[:])
                            nc.gpsimd.tensor_mul(t2[:], a[:, :, :, 1, :], cosb)
                            nc.gpsimd.tensor_mul(t3[:], a[:, :, :, 0, :], sinb)
                            nc.vector.tensor_add(rt[:, :, :, 1, :], t2[:], t3[:])
                            for ppend in pending:
                                ppend()
                            pending = []

                            def mk(rt_, slot_, t2p_):
                                def emit():
                                    h0 = SLOT_HEAD[slot_]
                                    # all 8 transposes of the batch into one
                                    # PSUM bank -> one wide copy + one DMA
                                    # with 512B lines
                                    pt = psT.tile([128, 4, 256], bf16, tag="psT")
                                    for ttl in range(2):
                                        for hh in range(4):
                                            nc.tensor.transpose(
                                                pt[:, hh, ts(ttl, 128)],
                                                rt_[:, ttl, hh].rearrange(
                                                    "p x j -> p (x j)"
                                                ),
                                                ident[:],
                                            )
                                    st = tstage_p.tile(
                                        [128, 4, 256], bf16, tag="ts"
                                    )
                                    nc.scalar.copy(out=st[:], in_=pt[:])
                                    nc.sync.dma_start(
                                        out=qkT_d[
                                            h0 : h0 + 4, :,
                                            128 * t2p_ : 128 * t2p_ + 256,
                                        ].rearrange("h p t -> p h t"),
                                        in_=st[:],
                                    )
                                return emit

                            pending.append(mk(rt, slot, t2p))
                        else:
                            vs = vstage_p.tile([128, 2, 512], bf16, tag="vs")
                            nc.scalar.copy(
                                out=vs[:], in_=acc[:, t2p : tt + 1, ts(j, 512)]
                            )
                            nc.sync.dma_start(
                                out=v_d[
                                    128 * t2p : 128 * (tt + 1), :
                                ].rearrange("(t p) f -> p t f", p=128),
                                in_=vs[:],
                            )
        # NOTE: the last rope batch's transposes stay in `pending`; they are
        # flushed after head 0's attention is emitted so the PE flows straight
        # from the last qkv matmul into score matmuls while the trailing rope
        # chain finishes on Pool/DVE.

        # ================= PHASE 2: attention ==============================
        with (
            tc.tile_pool(name="yt", bufs=NQ) as yt_p,
            tc.tile_pool(name="wp", bufs=2) as wp_p,
            tc.tile_pool(name="ostage", bufs=3) as ostage_p,
        ):
            yts = [
                yt_p.tile([128, T], bf16, tag="yt", name=f"yt{i}") for i in range(NQ)
            ]
            wps = {}

            with (
                tc.tile_pool(name="exp", bufs=5) as exp_p,
                tc.tile_pool(name="small", bufs=3) as small_p,
                tc.tile_pool(name="psS", bufs=3, space="PSUM") as psS,
                tc.tile_pool(name="psY", bufs=2, space="PSUM") as psY,
                tc.tile_pool(name="psD", bufs=2, space="PSUM") as psD,
            ):
                for g in range(NKV):
                    if g == 0:
                        kt = preload["kt0"]
                    else:
                        kt = kt_p.tile([128, T], bf16, tag="kt")
                        nc.sync.dma_start(out=kt[:], in_=qkT_d[NQ + g])
                    if g >= 1:
                        # prefetch the first Wproj block during attention,
                        # quartered to avoid head-of-line blocking qt loads
                        if g == 1:
                            wps[0] = wp_p.tile(
                                [128, 16, 512], bf16, tag="wp", name="wp0"
                            )
                        for qq in ([0, 1] if g == 1 else [2] if g == 2 else [3]):
                            nc.sync.dma_start(
                                out=wps[0][:, 4 * qq : 4 * (qq + 1), :],
                                in_=wproj_d[0, 4 * qq : 4 * (qq + 1)].rearrange(
                                    "y p f -> p y f"
                                ),
                            )
                    for r in range(REP):
                        hq = g * REP + r
                        # two-head lookahead on q loads
                        if hq + 2 < NQ and hq + 2 not in preload:
                            nxt = qt_p.tile(
                                [128, T], bf16, tag="qt", name=f"qt{hq + 2}"
                            )
                            nc.sync.dma_start(out=nxt[:], in_=qkT_d[hq + 2])
                            preload[hq + 2] = nxt
                        qt = preload.get(hq) or preload[f"qt{hq}"]

                        def emit_scores(chunk):
                            tq0 = 256 * chunk
                            npairs = chunk + 1
                            # scores + exp for every pair; the diagonal pair's
                            # second block is computed only for its valid
                            # upper t-half (compacted layout).
                            blocks = []  # (et, col0, width, si, py offset)
                            for pair in range(npairs):
                                si0 = 2 * pair
                                diag = pair == chunk
                                pss = psS.tile([128, 2, 256], f32, tag="psS")
                                flat = pss[:].rearrange("p a b -> p (a b)")
                                nc.tensor.matmul(
                                    pss[:, 0, :],
                                    kt[:, ts(si0, 128)],
                                    qt[:, tq0 : tq0 + 256],
                                    start=True,
                                    stop=True,
                                )
                                if not diag:
                                    nc.tensor.matmul(
                                        pss[:, 1, :],
                                        kt[:, ts(si0 + 1, 128)],
                                        qt[:, tq0 : tq0 + 256],
                                        start=True,
                                        stop=True,
                                    )
                                    et = exp_p.tile([128, 512], bf16, tag="exp")
                                    nc.scalar.activation(
                                        out=et[:], in_=flat, func=Exp, scale=SCALE
                                    )
                                    blocks.append((et, 0, 256, si0, 0))
                                    blocks.append((et, 256, 256, si0 + 1, 0))
                                else:
                                    nc.tensor.matmul(
                                        pss[:, 1, 0:128],
                                        kt[:, ts(si0 + 1, 128)],
                                        qt[:, tq0 + 128 : tq0 + 256],
                                        start=True,
                                        stop=True,
                                    )
                                    et = exp_p.tile([128, 384], bf16, tag="exp")
                                    nc.scalar.activation(
                                        out=et[:],
                                        in_=flat[:, 0:384],
                                        func=Exp,
                                        scale=SCALE,
                                    )
                                    nc.vector.tensor_mul(
                                        et[:], et[:], mask_sb[:]
                                    )
                                    blocks.append((et, 0, 256, si0, 0))
                                    blocks.append((et, 256, 128, si0 + 1, 128))
                            return blocks

                        def emit_av(chunk, blocks):
                            tq0 = 256 * chunk
                            ns = 2 * (chunk + 1)
                            # py = y accumulation; pd = softmax denominator
                            # (128-wide ones stationary puts the partition-sum
                            # on every output partition). Separate PSUM banks:
                            # interleaved accumulation groups must not share
                            # a bank.
                            py = psY.tile([128, 256], f32, tag="psY")
                            pd = psD.tile([128, 256], f32, tag="psD")
                            nb = len(blocks)
                            for b, (et, c0, w, si, off) in enumerate(blocks):
                                nc.tensor.matmul(
                                    py[:, off : off + w],
                                    v_sb[:, si, ts(g, 128)],
                                    et[:, c0 : c0 + w],
                                    start=(b == 0),
                                    stop=(b == nb - 1),
                                )
                                nc.tensor.matmul(
                                    pd[:, off : off + w],
                                    ones[:],
                                    et[:, c0 : c0 + w],
                                    start=(b == 0),
                                    stop=(b == nb - 1),
                                )
                            recip = small_p.tile([128, 256], f32, tag="recip")
                            nc.vector.reciprocal(out=recip[:], in_=pd[:])
                            nc.vector.tensor_mul(
                                yts[hq][:, tq0 : tq0 + 256], py[:], recip[:]
                            )

                        # software-pipeline: scores of chunk c+1 are emitted
                        # before the AV matmuls of chunk c so the in-order PE
                        # never waits on the Act exp chain
                        prev = None
                        for chunk in range(4):
                            blocks = emit_scores(chunk)
                            if prev is not None:
                                emit_av(*prev)
                            prev = (chunk, blocks)
                        emit_av(*prev)
                        if hq == 0:
                            for ppend in pending:
                                ppend()
                            pending = []

                # ============= PHASE 3: out = y @ Wproj (row shard) ============
                for ccol in range(8):
                    if ccol in wps:
                        wp = wps[ccol]
                    else:
                        wp = wp_p.tile([128, 16, 512], bf16, tag="wp")
                        nc.sync.dma_start(
                            out=wp[:, 0:8, :],
                            in_=wproj_d[ccol, 0:8].rearrange("y p f -> p y f"),
                        )
                        nc.sync.dma_start(
                            out=wp[:, 8:16, :],
                            in_=wproj_d[ccol, 8:16].rearrange("y p f -> p y f"),
                        )
                    for tt in range(NTT):
                        po = psS.tile([128, 512], f32, tag="psS")
                        for ycc in range(16):
                            nc.tensor.matmul(
                                po[:],
                                yts[ycc][:, ts(tt, 128)],
                                wp[:, ycc, :],
                                start=(ycc == 0),
                                stop=(ycc == 15),
                            )
                        ot = ostage_p.tile([128, 512], f32, tag="os")
                        if ccol == 7 and tt == NTT - 1:
                            # drain the last tile on two engines in parallel
                            nc.scalar.copy(out=ot[:, 0:256], in_=po[:, 0:256])
                            nc.vector.tensor_scalar_add(
                                ot[:, 256:512], po[:, 256:512], 0.0
                            )
                            nc.sync.dma_start(
                                out=out_d[ts(tt, 128), 512 * ccol : 512 * ccol + 256],
                                in_=ot[:, 0:256],
                            )
                            nc.sync.dma_start(
                                out=out_d[
                                    ts(tt, 128), 512 * ccol + 256 : 512 * ccol + 512
                                ],
                                in_=ot[:, 256:512],
                            )
                        else:
                            nc.scalar.copy(out=ot[:], in_=po[:])
                            nc.sync.dma_start(
                                out=out_d[ts(tt, 128), ts(ccol, 512)], in_=ot[:]
                            )

    nc.compile()
    return nc


def prep_inputs(x, Wqkv, Wproj, freqs_cos, freqs_sin):
    """Build the 8 per-core input maps (host-side shard + layout prep)."""
    x = np.asarray(x, np.float32)
    Wqkv = np.asarray(Wqkv, np.float32)
    Wproj = np.asarray(Wproj, np.float32)
    cos = np.ascontiguousarray(np.asarray(freqs_cos, np.float32))
    sin = np.ascontiguousarray(np.asarray(freqs_sin, np.float32))

    perm = np.concatenate([np.arange(0, HD, 2), np.arange(1, HD, 2)])
    p_ = np.arange(128)[:, None]
    masks = np.concatenate(
        [p_ <= np.arange(256)[None, :], p_ <= np.arange(128)[None, :]], axis=1
    ).astype(BF16)
    masks = np.ascontiguousarray(masks)

    in_maps = []
    for c in range(8):
        b, hh = divmod(c, 2)
        qcols = (hh * NQ * HD + (np.arange(NQ) * HD)[:, None] + perm[None, :]).ravel()
        kcols = (
            H * HD + hh * NKV * HD + (np.arange(NKV) * HD)[:, None] + perm[None, :]
        ).ravel()
        vcols = (
            (H + KV) * HD
            + hh * NKV * HD
            + (np.arange(NKV) * HD)[:, None]
            + np.arange(HD)[None, :]
        ).ravel()
        col_idx = np.concatenate([kcols, vcols, qcols])
        Wc = Wqkv[:, col_idx]  # [4096, 3072]
        wq = np.ascontiguousarray(
            Wc.reshape(2, 16, 128, 2, 3, 512).transpose(3, 0, 4, 1, 2, 5)
        ).astype(BF16)
        Wp = Wproj[hh * NQ * HD : (hh + 1) * NQ * HD, :]  # [2048, 4096]
        wp = np.ascontiguousarray(
            Wp.reshape(16, 128, 8, 512).transpose(2, 0, 1, 3)
        ).astype(BF16)
        xT = np.ascontiguousarray(x[b].T).astype(BF16)  # [4096, 1024]
        in_maps.append(
            {"xT": xT, "wqkv": wq, "wproj": wp, "cosn": cos, "sinn": sin,
             "masks": masks}
        )
    return in_maps


def _get_nc():
    if "nc" not in _CACHE:
        _CACHE["nc"] = _build_nc()
    return _CACHE["nc"]


def kernel(x, Wqkv, Wproj, freqs_cos, freqs_sin, mask=None):
    from concourse.bass_utils import run_bass_kernel_spmd

    nc = _get_nc()
    in_maps = prep_inputs(x, Wqkv, Wproj, freqs_cos, freqs_sin)
    res = run_bass_kernel_spmd(nc, in_maps, core_ids=list(range(8)))
    outs = [res.results[c]["out"] for c in range(8)]
    y = np.stack([outs[2 * b] + outs[2 * b + 1] for b in range(B)], axis=0)
    return y.astype(np.float32)
